# revision 21
# baseline (speedup 1.0000x reference)
"""Bidirectional batched GRU over ragged sequences on 8 Trainium2 NeuronCores.

Layout: hidden dim H=300 on partitions (3 chunks 128/128/44), batch on the
free dim. Per core: 256 segments, fwd+bwd scans interleaved. Biases enter via
an augmented ones-row in the matmul rhs. Outputs are PE-transposed (identity
matmul) and scattered to the flat ragged layout via indirect DMA.

v3: persistent PSUM accumulators (fine-grained instruction deps instead of
per-step pool-slot recycling), x-projection matmuls decoupled from the h
recurrence chain, 3-way split sigmoid, chunk-staged hidden update, bias folded
into the node tensor host-side. Segments are separated by 3 guard rows so one
single-offset indirect descriptor can move 4 consecutive positions (gather)
or 2 (scatter, into per-direction outputs concatenated host-side).
"""
import sys
sys.path.insert(0, "/opt/trn_rl_repo")
import numpy as np
import ml_dtypes

import concourse.bass as bass
import concourse.mybir as mybir
import concourse.tile as _tile_mod
from concourse.tile import TileContext
from concourse.tile_rust import add_dep_helper
from concourse.bass_utils import run_bass_kernel_spmd
from concourse.masks import make_identity

# ---- workaround: this walrus build rejects instructions carrying more than
# one semaphore wait. (a) distribute the TileContext tail-drain waits over
# single-wait SP no-ops; (b) post-pass hoisting excess waits anywhere else.
try:
    from bass_rust import ScopedClock as _ScopedClock
except ImportError:
    _ScopedClock = _tile_mod.ScopedClock


def _patched_drain_and_barrier(self, tick_clock, wait_clock):
    nc = self.nc
    probe = nc.sync.nop()
    wait_clock.add_sem_waits(probe.ins, _ScopedClock({None: tick_clock.global_clock}))
    si = probe.ins.sync_info
    waits = list(si.on_wait) if si is not None else []
    ups = list(si.on_update) if si is not None else []
    probe.ins.sync_info = mybir.SyncInfo(on_wait=[], on_update=ups)
    for w in waits:
        nc.sync.nop().ins.sync_info = mybir.SyncInfo(on_wait=[w], on_update=[])
    nc.sync.drain()
    nc.all_engine_barrier()
    assert self.sems is not None
    popped = nc._tile_sem_poison_stack.pop()
    assert popped is self._sem_poison
    nc.clear_and_free_semaphores(list(self.sems.allocated().values()))
    nc.all_engine_barrier()


TileContext._drain_and_barrier = _patched_drain_and_barrier
_nop_ctr = [0]


def _split_waits(nc, maxw=1):
    n_split = 0
    for fn in nc.m.functions:
        for bb in fn.blocks:
            il = bb.instructions
            newl = []
            for ins in il:
                si = ins.sync_info
                if si is not None and len(si.on_wait) > maxw:
                    waits = list(si.on_wait)
                    ups = list(si.on_update)
                    hoist, keep = waits[:-maxw], waits[-maxw:]
                    for i in range(0, len(hoist), maxw):
                        _nop_ctr[0] += 1
                        nop = mybir.InstNoOp(
                            name=f"waitnop-{_nop_ctr[0]}",
                            sync_info=mybir.SyncInfo(
                                on_wait=hoist[i:i + maxw], on_update=[]),
                            bass_nofuse=True,
                            engine=ins.engine)
                        nc.register_instruction(nop, overwrite=True)
                        newl.append(nop)
                    ins.sync_info = mybir.SyncInfo(on_wait=keep, on_update=ups)
                    n_split += 1
                newl.append(ins)
            il[:] = newl
    return n_split

F32 = mybir.dt.float32
BF16 = mybir.dt.bfloat16
I32 = mybir.dt.int32
AF = mybir.ActivationFunctionType
ALU = mybir.AluOpType

B, H, L, N = 2048, 300, 128, 131072
NCORES = 8
BC = B // NCORES          # 256 segments per core
H3 = 3 * H                # 900
HP = [(0, 128), (128, 256), (256, 300)]        # H chunks (partition dim)
# permuted gate-row order: [r0 r1 z0 z1 (r2 z2) n0 n1 n2]
PERM = np.concatenate([
    np.arange(0, 128), np.arange(128, 256),         # r0 r1
    np.arange(300, 428), np.arange(428, 556),       # z0 z1
    np.arange(256, 300), np.arange(556, 600),       # r2 z2  (Mc4, 88 rows)
    np.arange(600, 900),                            # n
])
MC = [(0, 128), (128, 256), (256, 384), (384, 512), (512, 600),
      (600, 728), (728, 856), (856, 900)]           # M chunks (permuted space)
OOB = 2 ** 30
GB = 4                    # gather batch (consecutive positions per descriptor)
GUARD = 3                 # guard rows between segments

_cache = {}


def _dep(after, before):
    # force same-engine stream order: `after` must issue after `before`
    add_dep_helper(after.ins, before.ins, sync=False,
                   reason="psum bank pending-zero order")


def _build(nc_cap):
    nc = bass.Bass()
    node = nc.dram_tensor("node", [nc_cap, H], F32, kind="ExternalInput")
    idx = nc.dram_tensor("idx", [BC, L], I32, kind="ExternalInput")
    negbv = nc.dram_tensor("negbv", [H, 1], F32, kind="ExternalInput")
    win = {}
    for d in range(2):
        for nm in ("wk0", "wk1", "hk0", "hk1"):
            win[(nm, d)] = nc.dram_tensor(f"{nm}_{d}", [128, H3], BF16,
                                          kind="ExternalInput")
        win[("k2m", d)] = nc.dram_tensor(f"k2m_{d}", [112, H3], BF16,
                                         kind="ExternalInput")
    cinit = nc.dram_tensor("cinit", [68, BC], BF16, kind="ExternalInput")
    msg = nc.dram_tensor("msg", [L, 128, 768], BF16)  # internal scratch
    out = nc.dram_tensor("out", [nc_cap, 2 * H], F32, kind="ExternalOutput")

    with TileContext(nc) as tc, \
         tc.tile_pool(name="persist", bufs=1) as pers:
        breg = nc.gpsimd.to_reg(nc_cap - 1)
        def ptile(shape, dtype, name):
            return pers.tile(shape, dtype, name=name, tag=name)
        consts = ptile([128, 128], F32, "consts")
        idf = consts[:, 0:128]
        make_identity(nc, idf)
        idb_t = ptile([128, 128], BF16, "idb")
        make_identity(nc, idb_t[:])
        negb_sb = ptile([128, 3], F32, "negb_sb")
        for c, (lo, hi) in enumerate(HP):
            nc.sync.dma_start(out=negb_sb[0:hi - lo, c:c + 1], in_=negbv[lo:hi, :])
        idx_sb = []
        for hh in range(2):
            t_ = ptile([128, L], I32, f"idx{hh}")
            nc.sync.dma_start(out=t_[:], in_=idx[hh * 128:(hh + 1) * 128, :])
            idx_sb.append(t_)
        # weights
        W = {}
        for d in range(2):
            for nm in ("wk0", "wk1", "hk0", "hk1"):
                t_ = ptile([128, H3], BF16, f"{nm}_{d}_sb")
                nc.sync.dma_start(out=t_[:], in_=win[(nm, d)][:])
                W[(nm, d)] = t_
            t_ = ptile([112, H3], BF16, f"k2m_{d}_sb")
            nc.sync.dma_start(out=t_[:], in_=win[("k2m", d)][:])
            W[("k2m", d)] = t_
        # persistent state
        h0acc = ptile([128, 768], F32, "h0acc")   # max(node+bias), chunk-major
        hk = {}   # (dir, chunk 0/1) -> [128, 256] bf16
        for d in range(2):
            for c in range(2):
                hk[(d, c)] = ptile([128, BC], BF16, f"h_{d}_{c}")
        comb = {}  # (dir, pingpong) -> [112, 256]: rows 0:44 h2, 44 ones, 64:108 x2, 108 ones
        for d in range(2):
            for pp in range(2):
                t_ = ptile([112, BC], BF16, f"comb_{d}_{pp}")
                nc.sync.dma_start(out=t_[44:112, :], in_=cinit[:])
                comb[(d, pp)] = t_

        # ---------------- Phase A: gather -> transpose -> relu -> msg slabs + h0
        with tc.tile_pool(name="gpool", bufs=3) as gpool, \
             tc.tile_pool(name="spsum", bufs=2, space="PSUM") as spsum, \
             tc.tile_pool(name="slabpool", bufs=3) as slabpool:
            for t in range(L):
                ps = spsum.tile([128, 768], F32, tag="ps")
                for hh in range(2):
                    g = gpool.tile([128, H], F32, tag=f"g{hh}")
                    nc.vector.memset(g[:], -1.0e30)
                    nc.gpsimd.indirect_dma_start(
                        out=g[:], out_offset=None, in_=node[:],
                        in_offset=bass.IndirectOffsetOnAxis(
                            ap=idx_sb[hh][:, t:t + 1], axis=0),
                        bounds_check=breg, oob_is_err=False)
                    for c, (lo, hi) in enumerate(HP):
                        nc.tensor.matmul(
                            out=ps[0:hi - lo, c * 256 + hh * 128: c * 256 + hh * 128 + 128],
                            lhsT=g[:, lo:hi], rhs=idf[0:128, 0:128],
                            start=True, stop=True)
                slab = slabpool.tile([128, 768], BF16, tag="slab_sb")
                nc.scalar.activation(out=slab[:], in_=ps[:], func=AF.Relu)
                if t == 0:
                    nc.vector.tensor_copy(out=h0acc[:], in_=ps[:])
                else:
                    nc.vector.tensor_tensor(out=h0acc[:], in0=h0acc[:],
                                            in1=ps[:], op=ALU.max)
                nc.sync.dma_start(out=msg[t], in_=slab[:])

        # h state init from h0acc (undo the folded +bias with ACT bias=-b)
        for d in range(2):
            for c in range(2):
                nc.scalar.activation(out=hk[(d, c)][:], in_=h0acc[:, c * 256:(c + 1) * 256],
                                     func=AF.Identity, bias=negb_sb[:, c:c + 1], scale=1.0)
            nc.scalar.activation(out=comb[(d, 0 if d == 0 else 1)][0:44, :],
                                 in_=h0acc[0:44, 512:768],
                                 func=AF.Identity, bias=negb_sb[0:44, 2:3], scale=1.0)

        # ---------------- Phase B: interleaved fwd/bwd scan
        with tc.tile_pool(name="pscan", bufs=1, space="PSUM") as pscan, \
             tc.tile_pool(name="xpool", bufs=4) as xpool, \
             tc.tile_pool(name="gates", bufs=2) as gates, \
             tc.tile_pool(name="opool", bufs=3) as opool:
            P = [pscan.tile([128, 2048], F32, name=f"P{d}", tag=f"P{d}")
                 for d in range(2)]
            # region -> psum column map: each bank holds one rz region (whose
            # first matmul start=True marks the bank's pending-zero bits) and
            # one nh region (start=False, riding the sibling's mark).
            PCOL_RZ = [0, 512, 1024, 1536, 1792]      # r0 r1 z0 z1 (r2z2)
            PCOL_NH = [256, 768, 1280]                # n0 n1 n2
            PCOL_XN = [0, 512, 1024]                  # xn overlays r0 r1 z0
            OUTCOL = 1280                             # out overlays n2 z1 E[0:88]
            for s in range(L):
                # Emission is phase-grouped across BOTH directions so that no
                # late-gated PE op (xn gated on sigmoid, transposes gated on
                # h') sits ahead of the other direction's ready matmuls in the
                # PE FIFO — otherwise the two recurrence chains serialize.
                ST = []
                for d in range(2):
                    t = s if d == 0 else L - 1 - s
                    st = {"t": t, "Pd": P[d], "cb": comb[(d, t % 2)],
                          "cbn": comb[(d, (t + 1) % 2)],
                          "wk0": W[("wk0", d)], "wk1": W[("wk1", d)],
                          "hk0w": W[("hk0", d)], "hk1w": W[("hk1", d)],
                          "k2m": W[("k2m", d)],
                          "h0t": hk[(d, 0)], "h1t": hk[(d, 1)]}
                    xk0 = xpool.tile([128, BC], BF16, tag=f"xk0{d}",
                                     name=f"xk0{d}_{s}")
                    xk1 = xpool.tile([128, BC], BF16, tag=f"xk1{d}",
                                     name=f"xk1{d}_{s}")
                    nc.sync.dma_start(out=xk0[:], in_=msg[t, :, 0:256])
                    nc.sync.dma_start(out=xk1[:], in_=msg[t, :, 256:512])
                    nc.sync.dma_start(out=st["cb"][64:108, :], in_=msg[t, 0:44, 512:768])
                    st["xk0"], st["xk1"] = xk0, xk1
                    ST.append(st)
                # --- phase 1: main matmul burst (x-side, h-side, nh), both dirs
                for st in ST:
                    Pd, cb = st["Pd"], st["cb"]
                    xk0, xk1 = st["xk0"], st["xk1"]
                    wk0, wk1, hk0w, hk1w, k2m = (st["wk0"], st["wk1"],
                                                 st["hk0w"], st["hk1w"], st["k2m"])
                    h0t, h1t = st["h0t"], st["h1t"]
                    g1first = {}
                    g1last = {}
                    for j in range(4):
                        lo, hi = MC[j]
                        m = hi - lo
                        o = Pd[0:m, PCOL_RZ[j]:PCOL_RZ[j] + BC]
                        mm = nc.tensor.matmul(out=o, lhsT=wk0[:, lo:hi], rhs=xk0[:],
                                              start=True, stop=False,
                                              skip_group_check=True)
                        g1first[j] = mm
                        nc.tensor.matmul(out=o, lhsT=wk1[:, lo:hi], rhs=xk1[:],
                                         start=False, stop=False,
                                         skip_group_check=True)
                    for j in range(4):
                        lo, hi = MC[j]
                        m = hi - lo
                        o = Pd[0:m, PCOL_RZ[j]:PCOL_RZ[j] + BC]
                        nc.tensor.matmul(out=o, lhsT=hk0w[:, lo:hi], rhs=h0t[:],
                                         start=False, stop=False,
                                         skip_group_check=True)
                        nc.tensor.matmul(out=o, lhsT=hk1w[:, lo:hi], rhs=h1t[:],
                                         start=False, stop=False,
                                         skip_group_check=True)
                        g1last[j] = nc.tensor.matmul(
                            out=o, lhsT=k2m[0:109, lo:hi], rhs=cb[0:109, :],
                            start=False, stop=True, skip_group_check=True)
                    # region E full group after z1's group (shared bank3)
                    lo, hi = MC[4]
                    m = hi - lo
                    o = Pd[0:m, PCOL_RZ[4]:PCOL_RZ[4] + BC]
                    mm = nc.tensor.matmul(out=o, lhsT=wk0[:, lo:hi], rhs=xk0[:],
                                          start=True, stop=False,
                                          skip_group_check=True)
                    g1first[4] = mm
                    _dep(mm, g1last[3])
                    nc.tensor.matmul(out=o, lhsT=wk1[:, lo:hi], rhs=xk1[:],
                                     start=False, stop=False, skip_group_check=True)
                    nc.tensor.matmul(out=o, lhsT=hk0w[:, lo:hi], rhs=h0t[:],
                                     start=False, stop=False, skip_group_check=True)
                    nc.tensor.matmul(out=o, lhsT=hk1w[:, lo:hi], rhs=h1t[:],
                                     start=False, stop=False, skip_group_check=True)
                    g1last[4] = nc.tensor.matmul(
                        out=o, lhsT=k2m[0:109, lo:hi], rhs=cb[0:109, :],
                        start=False, stop=True, skip_group_check=True)
                    g2last = {}
                    for jj in range(3):
                        lo, hi = MC[5 + jj]
                        m = hi - lo
                        o = Pd[0:m, PCOL_NH[jj]:PCOL_NH[jj] + BC]
                        mm = nc.tensor.matmul(out=o, lhsT=hk0w[:, lo:hi], rhs=h0t[:],
                                              start=False, stop=False,
                                              skip_group_check=True)
                        _dep(mm, g1first[jj])
                        nc.tensor.matmul(out=o, lhsT=hk1w[:, lo:hi], rhs=h1t[:],
                                         start=False, stop=False,
                                         skip_group_check=True)
                        g2last[jj] = nc.tensor.matmul(
                            out=o, lhsT=k2m[0:45, lo:hi], rhs=cb[0:45, :],
                            start=False, stop=True, skip_group_check=True)
                    st["g1last"], st["g2last"] = g1last, g2last
                # --- phase 2: sigmoids + z2 shift, both dirs
                for d, st in enumerate(ST):
                    Pd = st["Pd"]
                    rz = gates.tile([128, 1280], BF16, tag=f"rz{d}",
                                    name=f"rz{d}_{s}")
                    nc.scalar.activation(out=rz[:, 0:256], in_=Pd[:, 0:256],
                                         func=AF.Sigmoid)
                    nc.scalar.activation(out=rz[:, 256:512], in_=Pd[:, 512:768],
                                         func=AF.Sigmoid)
                    nc.scalar.activation(out=rz[:, 768:1280], in_=Pd[:, 1536:2048],
                                         func=AF.Sigmoid)
                    nc.scalar.activation(out=rz[:, 512:768], in_=Pd[:, 1024:1280],
                                         func=AF.Sigmoid)
                    z2c = gates.tile([44, BC], BF16, tag=f"z2{d}",
                                     name=f"z2{d}_{s}")
                    nc.sync.dma_start(out=z2c[0:44, :], in_=rz[44:88, 1024:1280])
                    st["rz"], st["z2c"] = rz, z2c
                # --- phase 3: xn matmuls, both dirs
                for st in ST:
                    Pd, cb = st["Pd"], st["cb"]
                    xk0, xk1, k2m = st["xk0"], st["xk1"], st["k2m"]
                    wk0, wk1 = st["wk0"], st["wk1"]
                    xn_last = {}
                    for jj in range(3):
                        lo, hi = MC[5 + jj]
                        m = hi - lo
                        o = Pd[0:m, PCOL_XN[jj]:PCOL_XN[jj] + BC]
                        mm = nc.tensor.matmul(out=o, lhsT=wk0[:, lo:hi], rhs=xk0[:],
                                              start=True, stop=False,
                                              skip_group_check=True)
                        _dep(mm, st["g1last"][jj])
                        _dep(mm, st["g2last"][jj])
                        nc.tensor.matmul(out=o, lhsT=wk1[:, lo:hi], rhs=xk1[:],
                                         start=False, stop=False,
                                         skip_group_check=True)
                        xn_last[jj] = nc.tensor.matmul(
                            out=o, lhsT=k2m[64:109, lo:hi], rhs=cb[64:109, :],
                            start=False, stop=True, skip_group_check=True)
                    st["xn_last"] = xn_last
                # --- phase 4: tmp/ssb/tanh, both dirs
                for d, st in enumerate(ST):
                    Pd, rz = st["Pd"], st["rz"]
                    tmp = gates.tile([128, 768], F32, tag=f"tmp{d}",
                                     name=f"tmp{d}_{s}")
                    rsl = [rz[:, 0:256], rz[:, 256:512], rz[0:44, 1024:1280]]
                    for c, (lo, hi) in enumerate(HP):
                        r = hi - lo
                        nc.vector.tensor_tensor(
                            out=tmp[0:r, c * 256:(c + 1) * 256], in0=rsl[c],
                            in1=Pd[0:r, PCOL_NH[c]:PCOL_NH[c] + BC],
                            op=ALU.mult)
                    ssb = gates.tile([128, 768], F32, tag=f"s{d}",
                                     name=f"s{d}_{s}")
                    for c, (lo, hi) in enumerate(HP):
                        r = hi - lo
                        nc.vector.tensor_tensor(
                            out=ssb[0:r, c * 256:(c + 1) * 256],
                            in0=tmp[0:r, c * 256:(c + 1) * 256],
                            in1=Pd[0:r, PCOL_XN[c]:PCOL_XN[c] + BC],
                            op=ALU.add)
                    nsb = gates.tile([128, 768], BF16, tag=f"n{d}",
                                     name=f"n{d}_{s}")
                    nc.scalar.activation(out=nsb[:], in_=ssb[:], func=AF.Tanh)
                    st["nsb"] = nsb
                # --- phase 5: h' = n + z*(h-n), both dirs
                for d, st in enumerate(ST):
                    rz, z2c, nsb = st["rz"], st["z2c"], st["nsb"]
                    h0t, h1t, cb, cbn = st["h0t"], st["h1t"], st["cb"], st["cbn"]
                    zsl = [rz[:, 512:768], rz[:, 768:1024], z2c[0:44, :]]
                    hsl = [h0t[:], h1t[:], cb[0:44, :]]
                    hnx = [h0t[:], h1t[:], cbn[0:44, :]]
                    for c, (lo, hi) in enumerate(HP):
                        r = hi - lo
                        ns = nsb[0:r, c * 256:(c + 1) * 256]
                        dd = gates.tile([128, BC], BF16, tag=f"d{d}",
                                        name=f"d{d}_{s}_{c}")
                        nc.vector.tensor_tensor(out=dd[0:r, :],
                                                in0=hsl[c][0:r, :] if c == 2 else hsl[c],
                                                in1=ns, op=ALU.subtract)
                        ee = gates.tile([128, BC], BF16, tag=f"e{d}",
                                        name=f"e{d}_{s}_{c}")
                        eng = nc.vector if c == 0 else nc.gpsimd
                        eng.tensor_tensor(out=ee[0:r, :],
                                          in0=zsl[c][0:r, :] if c == 2 else zsl[c],
                                          in1=dd[0:r, :], op=ALU.mult)
                        nc.vector.tensor_tensor(out=hnx[c][0:r, :] if c == 2 else hnx[c],
                                                in0=ns, in1=ee[0:r, :], op=ALU.add)
                # --- phase 6: transposes + out copies, both dirs
                scatters = []
                for d, st in enumerate(ST):
                    Pd = st["Pd"]
                    hpieces = [st["h0t"], st["h1t"], st["cbn"]]
                    for bh in range(2):
                        for c, (lo, hi) in enumerate(HP):
                            r = hi - lo
                            src = hpieces[c]
                            lhsT = (src[0:44, bh * 128:(bh + 1) * 128] if c == 2
                                    else src[:, bh * 128:(bh + 1) * 128])
                            trmm = nc.tensor.matmul(
                                out=Pd[:, 1280 + bh * 300 + lo:1280 + bh * 300 + hi],
                                lhsT=lhsT, rhs=idb_t[0:r, 0:r],
                                start=True, stop=True, skip_group_check=True)
                            if bh == 0 and c <= 1:
                                # bank2 pending-mark must not precede xn2's
                                # accumulation (no AP overlap to order them)
                                _dep(trmm, st["xn_last"][2])
                        osb = opool.tile([128, H], F32, tag="osb",
                                         name=f"osb{d}_{s}_{bh}")
                        nc.scalar.activation(out=osb[:],
                                             in_=Pd[:, 1280 + bh * 300:1280 + bh * 300 + 300],
                                             func=AF.Copy)
                        scatters.append((d, st["t"], bh, osb))
                # scatters last so the GpSimd FIFO head never blocks updates
                for d, tt_, bh, osb in scatters:
                    nc.gpsimd.indirect_dma_start(
                        out=out[:, :],
                        out_offset=bass.IndirectOffsetOnAxis(
                            ap=idx_sb[bh][:, tt_:tt_ + 1], axis=0),
                        in_=osb[:, :], in_offset=None,
                        element_offset=d * H,
                        bounds_check=breg, oob_is_err=False)

    _split_waits(nc)
    return nc


def _prep_weights(w_ih, w_hh, b_ih, b_hh):
    wT = np.ascontiguousarray(w_ih[PERM, :].T)          # [300, 900]
    hT = np.ascontiguousarray(w_hh[PERM, :].T)
    bi = b_ih[PERM]
    bh = b_hh[PERM]
    n_mask = PERM >= 600
    aug_x = np.where(n_mask, bi, 0.0)                   # b_ih for n via x ones-row
    aug_h = bh + np.where(~n_mask, bi, 0.0)             # b_hh (+ b_ih for r,z)
    bf = ml_dtypes.bfloat16
    return {
        "wk0": np.ascontiguousarray(wT[0:128]).astype(bf),
        "wk1": np.ascontiguousarray(wT[128:256]).astype(bf),
        "hk0": np.ascontiguousarray(hT[0:128]).astype(bf),
        "hk1": np.ascontiguousarray(hT[128:256]).astype(bf),
        "k2m": np.ascontiguousarray(
            np.vstack([hT[256:300], aug_h[None, :], np.zeros((19, H3), np.float32),
                       wT[256:300], aug_x[None, :], np.zeros((3, H3), np.float32)])
        ).astype(bf),
    }


TRACE = False
TRACE_DIR = None
LAST_EXEC_NS = None


def kernel(node, a_scope, max_len, bias, w_ih_f, w_hh_f, b_ih_f, b_hh_f,
           w_ih_b, w_hh_b, b_ih_b, b_hh_b):
    global LAST_EXEC_NS
    node = np.asarray(node, dtype=np.float32)
    bias_np = np.asarray(bias, dtype=np.float32)
    a_scope = np.asarray(a_scope, dtype=np.int64)
    assert int(max_len) == L and node.shape == (N, H) and a_scope.shape == (B,)

    ends = np.cumsum(a_scope)
    starts = ends - a_scope
    core_lo = starts[0::BC]                    # first row of each core's block
    core_hi = ends[BC - 1::BC]                 # end row of each core's block
    rows = (core_hi - core_lo).astype(np.int64)
    nc_cap = int(rows.max())

    key = nc_cap
    if key not in _cache:
        _cache[key] = _build(nc_cap)
    nc = _cache[key]

    wf = _prep_weights(np.asarray(w_ih_f), np.asarray(w_hh_f),
                       np.asarray(b_ih_f), np.asarray(b_hh_f))
    wb = _prep_weights(np.asarray(w_ih_b), np.asarray(w_hh_b),
                       np.asarray(b_ih_b), np.asarray(b_hh_b))
    negb = np.ascontiguousarray(-bias_np.reshape(H, 1))
    node_b = node + bias_np[None, :]           # fold bias host-side

    in_maps = []
    for c in range(NCORES):
        lo, hi = int(core_lo[c]), int(core_hi[c])
        node_c = np.zeros((nc_cap, H), dtype=np.float32)
        node_c[:hi - lo] = node_b[lo:hi]
        st = (starts[c * BC:(c + 1) * BC] - lo).astype(np.int64)
        ln = a_scope[c * BC:(c + 1) * BC]
        tt = np.arange(L, dtype=np.int64)
        im = st[:, None] + tt[None, :]                      # [BC, L]
        im = np.where(tt[None, :] < ln[:, None], im, OOB).astype(np.int32)
        ci = np.zeros((68, BC), dtype=ml_dtypes.bfloat16)
        ci[0] = 1.0   # comb row 44: ones (aug_h)
        ci[64] = 1.0  # comb row 108: ones (aug_x)
        m = {"node": node_c, "idx": np.ascontiguousarray(im),
             "negbv": negb, "cinit": ci}
        for d, wd in enumerate((wf, wb)):
            for nm in ("wk0", "wk1", "hk0", "hk1", "k2m"):
                m[f"{nm}_{d}"] = wd[nm]
        in_maps.append(m)

    res = run_bass_kernel_spmd(nc, in_maps, core_ids=list(range(NCORES)),
                               trace=TRACE, tmpdir=TRACE_DIR)
    LAST_EXEC_NS = res.exec_time_ns
    out = np.empty((N, 2 * H), dtype=np.float32)
    for c in range(NCORES):
        lo, hi = int(core_lo[c]), int(core_hi[c])
        out[lo:hi] = res.results[c]["out"][:hi - lo]
    return out


# revision 25
# speedup vs baseline: 1.1602x; 1.1602x over previous
"""Bidirectional batched GRU over ragged sequences on 8 Trainium2 NeuronCores.

Layout: hidden dim H=300 on partitions (3 chunks 128/128/44), batch on the
free dim. Per core: 256 segments, fwd+bwd scans interleaved. Biases enter via
an augmented ones-row in the matmul rhs. Outputs are PE-transposed (identity
matmul) and scattered to the flat ragged layout via indirect DMA.

v3: persistent PSUM accumulators (fine-grained instruction deps instead of
per-step pool-slot recycling), x-projection matmuls decoupled from the h
recurrence chain, 3-way split sigmoid, chunk-staged hidden update, bias folded
into the node tensor host-side. Segments are separated by 3 guard rows so one
single-offset indirect descriptor can move 4 consecutive positions (gather)
or 2 (scatter, into per-direction outputs concatenated host-side).
"""
import sys
sys.path.insert(0, "/opt/trn_rl_repo")
import numpy as np
import ml_dtypes

import concourse.bass as bass
import concourse.mybir as mybir
import concourse.tile as _tile_mod
from concourse.tile import TileContext
from concourse.tile_rust import add_dep_helper
from concourse.bass_utils import run_bass_kernel_spmd
from concourse.masks import make_identity

# ---- workaround: this walrus build rejects instructions carrying more than
# one semaphore wait. (a) distribute the TileContext tail-drain waits over
# single-wait SP no-ops; (b) post-pass hoisting excess waits anywhere else.
try:
    from bass_rust import ScopedClock as _ScopedClock
except ImportError:
    _ScopedClock = _tile_mod.ScopedClock


def _patched_drain_and_barrier(self, tick_clock, wait_clock):
    nc = self.nc
    probe = nc.sync.nop()
    wait_clock.add_sem_waits(probe.ins, _ScopedClock({None: tick_clock.global_clock}))
    si = probe.ins.sync_info
    waits = list(si.on_wait) if si is not None else []
    ups = list(si.on_update) if si is not None else []
    probe.ins.sync_info = mybir.SyncInfo(on_wait=[], on_update=ups)
    for w in waits:
        nc.sync.nop().ins.sync_info = mybir.SyncInfo(on_wait=[w], on_update=[])
    nc.sync.drain()
    nc.all_engine_barrier()
    assert self.sems is not None
    popped = nc._tile_sem_poison_stack.pop()
    assert popped is self._sem_poison
    nc.clear_and_free_semaphores(list(self.sems.allocated().values()))
    nc.all_engine_barrier()


TileContext._drain_and_barrier = _patched_drain_and_barrier
_nop_ctr = [0]


def _split_waits(nc, maxw=1):
    n_split = 0
    for fn in nc.m.functions:
        for bb in fn.blocks:
            il = bb.instructions
            newl = []
            for ins in il:
                si = ins.sync_info
                if si is not None and len(si.on_wait) > maxw:
                    waits = list(si.on_wait)
                    ups = list(si.on_update)
                    hoist, keep = waits[:-maxw], waits[-maxw:]
                    for i in range(0, len(hoist), maxw):
                        _nop_ctr[0] += 1
                        nop = mybir.InstNoOp(
                            name=f"waitnop-{_nop_ctr[0]}",
                            sync_info=mybir.SyncInfo(
                                on_wait=hoist[i:i + maxw], on_update=[]),
                            bass_nofuse=True,
                            engine=ins.engine)
                        nc.register_instruction(nop, overwrite=True)
                        newl.append(nop)
                    ins.sync_info = mybir.SyncInfo(on_wait=keep, on_update=ups)
                    n_split += 1
                newl.append(ins)
            il[:] = newl
    return n_split

F32 = mybir.dt.float32
BF16 = mybir.dt.bfloat16
I32 = mybir.dt.int32
AF = mybir.ActivationFunctionType
ALU = mybir.AluOpType

B, H, L, N = 2048, 300, 128, 131072
NCORES = 8
BC = B // NCORES          # 256 segments per core
H3 = 3 * H                # 900
HP = [(0, 128), (128, 256), (256, 300)]        # H chunks (partition dim)
# permuted gate-row order: [r0 r1 z0 z1 (r2 z2) n0 n1 n2]
PERM = np.concatenate([
    np.arange(0, 128), np.arange(128, 256),         # r0 r1
    np.arange(300, 428), np.arange(428, 556),       # z0 z1
    np.arange(256, 300), np.arange(556, 600),       # r2 z2  (Mc4, 88 rows)
    np.arange(600, 900),                            # n
])
MC = [(0, 128), (128, 256), (256, 384), (384, 512), (512, 600),
      (600, 728), (728, 856), (856, 900)]           # M chunks (permuted space)
OOB = 2 ** 30
GB = 4                    # gather batch (consecutive positions per descriptor)
GUARD = 3                 # guard rows between segments

_cache = {}


def _dep(after, before):
    # force same-engine stream order: `after` must issue after `before`
    add_dep_helper(after.ins, before.ins, sync=False,
                   reason="psum bank pending-zero order")


def _build(nc_cap):
    nc = bass.Bass()
    node = nc.dram_tensor("node", [nc_cap, H], F32, kind="ExternalInput")
    idx = nc.dram_tensor("idx", [BC, L], I32, kind="ExternalInput")
    negbv = nc.dram_tensor("negbv", [H, 1], F32, kind="ExternalInput")
    win = {}
    for d in range(2):
        for nm in ("wk0", "wk1", "hk0", "hk1"):
            win[(nm, d)] = nc.dram_tensor(f"{nm}_{d}", [128, H3], BF16,
                                          kind="ExternalInput")
        win[("k2m", d)] = nc.dram_tensor(f"k2m_{d}", [112, H3], BF16,
                                         kind="ExternalInput")
    cinit = nc.dram_tensor("cinit", [68, BC], BF16, kind="ExternalInput")
    msg = nc.dram_tensor("msg", [L, 128, 768], BF16)  # internal scratch
    out = nc.dram_tensor("out", [nc_cap, 2 * H], F32, kind="ExternalOutput")

    with TileContext(nc) as tc, \
         tc.tile_pool(name="persist", bufs=1) as pers:
        breg = nc.gpsimd.to_reg(nc_cap - 1)
        def ptile(shape, dtype, name):
            return pers.tile(shape, dtype, name=name, tag=name)
        consts = ptile([128, 128], F32, "consts")
        idf = consts[:, 0:128]
        make_identity(nc, idf)
        idb_t = ptile([128, 128], BF16, "idb")
        make_identity(nc, idb_t[:])
        negb_sb = ptile([128, 3], F32, "negb_sb")
        for c, (lo, hi) in enumerate(HP):
            nc.sync.dma_start(out=negb_sb[0:hi - lo, c:c + 1], in_=negbv[lo:hi, :])
        idx_sb = []
        for hh in range(2):
            t_ = ptile([128, L], I32, f"idx{hh}")
            nc.sync.dma_start(out=t_[:], in_=idx[hh * 128:(hh + 1) * 128, :])
            idx_sb.append(t_)
        # weights
        W = {}
        for d in range(2):
            for nm in ("wk0", "wk1", "hk0", "hk1"):
                t_ = ptile([128, H3], BF16, f"{nm}_{d}_sb")
                nc.sync.dma_start(out=t_[:], in_=win[(nm, d)][:])
                W[(nm, d)] = t_
            t_ = ptile([112, H3], BF16, f"k2m_{d}_sb")
            nc.sync.dma_start(out=t_[:], in_=win[("k2m", d)][:])
            W[("k2m", d)] = t_
        # persistent state
        h0acc = ptile([128, 768], F32, "h0acc")   # max(node+bias), chunk-major
        hk = {}   # (dir, chunk 0/1) -> [128, 256] bf16
        for d in range(2):
            for c in range(2):
                hk[(d, c)] = ptile([128, BC], BF16, f"h_{d}_{c}")
        comb = {}  # (dir, pingpong) -> [112, 256]: rows 0:44 h2, 44 ones, 64:108 x2, 108 ones
        for d in range(2):
            for pp in range(2):
                t_ = ptile([112, BC], BF16, f"comb_{d}_{pp}")
                nc.sync.dma_start(out=t_[44:112, :], in_=cinit[:])
                comb[(d, pp)] = t_

        # ---------------- Phase A: gather -> transpose -> relu -> msg slabs + h0
        with tc.tile_pool(name="gpool", bufs=4) as gpool, \
             tc.tile_pool(name="spsum", bufs=3, space="PSUM") as spsum, \
             tc.tile_pool(name="slabpool", bufs=3) as slabpool:
            for t in range(L):
                ps = spsum.tile([128, 768], F32, tag="ps")
                for hh in range(2):
                    g = gpool.tile([128, H], F32, tag=f"g{hh}")
                    nc.vector.memset(g[:], -1.0e30)
                    nc.gpsimd.indirect_dma_start(
                        out=g[:], out_offset=None, in_=node[:],
                        in_offset=bass.IndirectOffsetOnAxis(
                            ap=idx_sb[hh][:, t:t + 1], axis=0),
                        bounds_check=breg, oob_is_err=False)
                    for c, (lo, hi) in enumerate(HP):
                        nc.tensor.matmul(
                            out=ps[0:hi - lo, c * 256 + hh * 128: c * 256 + hh * 128 + 128],
                            lhsT=g[:, lo:hi], rhs=idf[0:128, 0:128],
                            start=True, stop=True)
                slab = slabpool.tile([128, 768], BF16, tag="slab_sb")
                nc.scalar.activation(out=slab[:], in_=ps[:], func=AF.Relu)
                if t == 0:
                    nc.vector.tensor_copy(out=h0acc[:], in_=ps[:])
                else:
                    nc.vector.tensor_tensor(out=h0acc[:], in0=h0acc[:],
                                            in1=ps[:], op=ALU.max)
                nc.sync.dma_start(out=msg[t], in_=slab[:])

        # h state init from h0acc (undo the folded +bias with ACT bias=-b)
        for d in range(2):
            for c in range(2):
                nc.scalar.activation(out=hk[(d, c)][:], in_=h0acc[:, c * 256:(c + 1) * 256],
                                     func=AF.Identity, bias=negb_sb[:, c:c + 1], scale=1.0)
            nc.scalar.activation(out=comb[(d, 0 if d == 0 else 1)][0:44, :],
                                 in_=h0acc[0:44, 512:768],
                                 func=AF.Identity, bias=negb_sb[0:44, 2:3], scale=1.0)

        # ---------------- Phase B: interleaved fwd/bwd scan
        with tc.tile_pool(name="pscan", bufs=1, space="PSUM") as pscan, \
             tc.tile_pool(name="xpool", bufs=4) as xpool, \
             tc.tile_pool(name="gates", bufs=3) as gates, \
             tc.tile_pool(name="opool", bufs=4) as opool:
            P = [pscan.tile([128, 2048], F32, name=f"P{d}", tag=f"P{d}")
                 for d in range(2)]
            # region -> psum column map: each bank holds one rz region (whose
            # first matmul start=True marks the bank's pending-zero bits) and
            # one nh region (start=False, riding the sibling's mark).
            PCOL_RZ = [0, 512, 1024, 1536, 1792]      # r0 r1 z0 z1 (r2z2)
            PCOL_NH = [256, 768, 1280]                # n0 n1 n2
            PCOL_XN = [0, 512, 1024]                  # xn overlays r0 r1 z0
            OUTCOL = 1280                             # out overlays n2 z1 E[0:88]
            for s in range(L):
                scatters = []
                for d in range(2):
                    t = s if d == 0 else L - 1 - s
                    Pd = P[d]
                    cb = comb[(d, t % 2)]
                    cbn = comb[(d, (t + 1) % 2)]
                    xk0 = xpool.tile([128, BC], BF16, tag="xk0")
                    xk1 = xpool.tile([128, BC], BF16, tag="xk1")
                    nc.sync.dma_start(out=xk0[:], in_=msg[t, :, 0:256])
                    nc.sync.dma_start(out=xk1[:], in_=msg[t, :, 256:512])
                    nc.sync.dma_start(out=cb[64:108, :], in_=msg[t, 0:44, 512:768])
                    wk0, wk1 = W[("wk0", d)], W[("wk1", d)]
                    hk0w, hk1w = W[("hk0", d)], W[("hk1", d)]
                    k2m = W[("k2m", d)]
                    h0t, h1t = hk[(d, 0)], hk[(d, 1)]
                    # --- x-side rz accumulation (start=True marks each bank).
                    # E (j=4) shares bank3 with z1 (j=3): its whole group is
                    # emitted after z1's group (bank writes execute in program
                    # order), so its bank-remark can't wipe z1 mid-group.
                    g1first = {}
                    g1last = {}
                    for j in range(4):
                        lo, hi = MC[j]
                        m = hi - lo
                        o = Pd[0:m, PCOL_RZ[j]:PCOL_RZ[j] + BC]
                        mm = nc.tensor.matmul(out=o, lhsT=wk0[:, lo:hi], rhs=xk0[:],
                                              start=True, stop=False,
                                              skip_group_check=True)
                        g1first[j] = mm
                        nc.tensor.matmul(out=o, lhsT=wk1[:, lo:hi], rhs=xk1[:],
                                         start=False, stop=False,
                                         skip_group_check=True)
                    # --- h-side rz accumulation
                    for j in range(4):
                        lo, hi = MC[j]
                        m = hi - lo
                        o = Pd[0:m, PCOL_RZ[j]:PCOL_RZ[j] + BC]
                        nc.tensor.matmul(out=o, lhsT=hk0w[:, lo:hi], rhs=h0t[:],
                                         start=False, stop=False,
                                         skip_group_check=True)
                        nc.tensor.matmul(out=o, lhsT=hk1w[:, lo:hi], rhs=h1t[:],
                                         start=False, stop=False,
                                         skip_group_check=True)
                        g1last[j] = nc.tensor.matmul(
                            out=o, lhsT=k2m[0:109, lo:hi], rhs=cb[0:109, :],
                            start=False, stop=True, skip_group_check=True)
                    # region E full group after z1's group
                    lo, hi = MC[4]
                    m = hi - lo
                    o = Pd[0:m, PCOL_RZ[4]:PCOL_RZ[4] + BC]
                    mm = nc.tensor.matmul(out=o, lhsT=wk0[:, lo:hi], rhs=xk0[:],
                                          start=True, stop=False,
                                          skip_group_check=True)
                    g1first[4] = mm
                    _dep(mm, g1last[3])
                    nc.tensor.matmul(out=o, lhsT=wk1[:, lo:hi], rhs=xk1[:],
                                     start=False, stop=False, skip_group_check=True)
                    nc.tensor.matmul(out=o, lhsT=hk0w[:, lo:hi], rhs=h0t[:],
                                     start=False, stop=False, skip_group_check=True)
                    nc.tensor.matmul(out=o, lhsT=hk1w[:, lo:hi], rhs=h1t[:],
                                     start=False, stop=False, skip_group_check=True)
                    g1last[4] = nc.tensor.matmul(
                        out=o, lhsT=k2m[0:109, lo:hi], rhs=cb[0:109, :],
                        start=False, stop=True, skip_group_check=True)
                    # --- nh regions ride their bank-sibling rz region's mark
                    g2first = {}
                    g2last = {}
                    for jj in range(3):
                        lo, hi = MC[5 + jj]
                        m = hi - lo
                        o = Pd[0:m, PCOL_NH[jj]:PCOL_NH[jj] + BC]
                        mm = nc.tensor.matmul(out=o, lhsT=hk0w[:, lo:hi], rhs=h0t[:],
                                              start=False, stop=False,
                                              skip_group_check=True)
                        g2first[jj] = mm
                        _dep(mm, g1first[jj])
                        nc.tensor.matmul(out=o, lhsT=hk1w[:, lo:hi], rhs=h1t[:],
                                         start=False, stop=False,
                                         skip_group_check=True)
                        g2last[jj] = nc.tensor.matmul(
                            out=o, lhsT=k2m[0:45, lo:hi], rhs=cb[0:45, :],
                            start=False, stop=True, skip_group_check=True)
                    # --- gates: split sigmoid so r0/r1/(r2z2) unblock tmp
                    # early. rz sbuf layout: r0 r1 z0 z1 E at 256-col slots.
                    rz = gates.tile([128, 1280], BF16, tag=f"rz{d}")
                    # order: r0 first (tmp chunk0), z0 second (unblocks xn2 →
                    # ssb), then r1, then z1+E merged
                    nc.scalar.activation(out=rz[:, 0:256], in_=Pd[:, 0:256],
                                         func=AF.Sigmoid)
                    nc.scalar.activation(out=rz[:, 512:768], in_=Pd[:, 1024:1280],
                                         func=AF.Sigmoid)
                    nc.scalar.activation(out=rz[:, 256:512], in_=Pd[:, 512:768],
                                         func=AF.Sigmoid)
                    nc.scalar.activation(out=rz[:, 768:1280], in_=Pd[:, 1536:2048],
                                         func=AF.Sigmoid)
                    # z2 partition shift 44:88 -> 0:44 (E sbuf slot is 1024:1280)
                    z2c = gates.tile([44, BC], BF16, tag=f"z2{d}")
                    nc.sync.dma_start(out=z2c[0:44, :], in_=rz[44:88, 1024:1280])
                    # --- xn into freed rz regions (re-marks banks 0,1,2 after
                    # both sibling groups completed)
                    xn_last = {}
                    for jj in range(3):
                        lo, hi = MC[5 + jj]
                        m = hi - lo
                        o = Pd[0:m, PCOL_XN[jj]:PCOL_XN[jj] + BC]
                        mm = nc.tensor.matmul(out=o, lhsT=wk0[:, lo:hi], rhs=xk0[:],
                                              start=True, stop=False,
                                              skip_group_check=True)
                        _dep(mm, g1last[jj])
                        _dep(mm, g2last[jj])
                        nc.tensor.matmul(out=o, lhsT=wk1[:, lo:hi], rhs=xk1[:],
                                         start=False, stop=False,
                                         skip_group_check=True)
                        xn_last[jj] = nc.tensor.matmul(
                            out=o, lhsT=k2m[64:109, lo:hi], rhs=cb[64:109, :],
                            start=False, stop=True, skip_group_check=True)
                    # tmp = r * nh ; s = tmp + xn ; n = tanh(s)
                    tmp = gates.tile([128, 768], F32, tag=f"tmp{d}")
                    rsl = [rz[:, 0:256], rz[:, 256:512], rz[0:44, 1024:1280]]
                    for c, (lo, hi) in enumerate(HP):
                        r = hi - lo
                        nc.vector.tensor_tensor(
                            out=tmp[0:r, c * 256:(c + 1) * 256], in0=rsl[c],
                            in1=Pd[0:r, PCOL_NH[c]:PCOL_NH[c] + BC],
                            op=ALU.mult)
                    ssb = gates.tile([128, 768], F32, tag=f"s{d}")
                    for c, (lo, hi) in enumerate(HP):
                        r = hi - lo
                        nc.vector.tensor_tensor(
                            out=ssb[0:r, c * 256:(c + 1) * 256],
                            in0=tmp[0:r, c * 256:(c + 1) * 256],
                            in1=Pd[0:r, PCOL_XN[c]:PCOL_XN[c] + BC],
                            op=ALU.add)
                    nsb = gates.tile([128, 768], BF16, tag=f"n{d}")
                    nc.scalar.activation(out=nsb[:], in_=ssb[:], func=AF.Tanh)
                    # h' = n + z*(h-n), chunk-staged (chunk 0 fully on DVE so
                    # the next step's hk0 matmuls unblock earliest)
                    zsl = [rz[:, 512:768], rz[:, 768:1024], z2c[0:44, :]]
                    hsl = [h0t[:], h1t[:], cb[0:44, :]]
                    hnx = [h0t[:], h1t[:], cbn[0:44, :]]
                    for c, (lo, hi) in enumerate(HP):
                        r = hi - lo
                        ns = nsb[0:r, c * 256:(c + 1) * 256]
                        dd = gates.tile([128, BC], BF16, tag=f"d{d}")
                        nc.vector.tensor_tensor(out=dd[0:r, :],
                                                in0=hsl[c][0:r, :] if c == 2 else hsl[c],
                                                in1=ns, op=ALU.subtract)
                        ee = gates.tile([128, BC], BF16, tag=f"e{d}")
                        eng = nc.vector if c == 0 else nc.gpsimd
                        eng.tensor_tensor(out=ee[0:r, :],
                                          in0=zsl[c][0:r, :] if c == 2 else zsl[c],
                                          in1=dd[0:r, :], op=ALU.mult)
                        nc.vector.tensor_tensor(out=hnx[c][0:r, :] if c == 2 else hnx[c],
                                                in0=ns, in1=ee[0:r, :], op=ALU.add)
                    # transpose h' to [b, H] in psum cols 1280:1880, write into
                    # the 2-step osb pair buffer; scatter fires every 2nd step
                    hpieces = [h0t, h1t, cbn]
                    for bh in range(2):
                        for c, (lo, hi) in enumerate(HP):
                            r = hi - lo
                            src = hpieces[c]
                            lhsT = (src[0:44, bh * 128:(bh + 1) * 128] if c == 2
                                    else src[:, bh * 128:(bh + 1) * 128])
                            trmm = nc.tensor.matmul(
                                out=Pd[:, 1280 + bh * 300 + lo:1280 + bh * 300 + hi],
                                lhsT=lhsT, rhs=idb_t[0:r, 0:r],
                                start=True, stop=True, skip_group_check=True)
                            if bh == 0 and c <= 1:
                                # bank2 pending-mark must not precede xn2's
                                # accumulation (no AP overlap to order them)
                                _dep(trmm, xn_last[2])
                        osb = opool.tile([128, H], F32, tag="osb")
                        nc.scalar.activation(out=osb[:],
                                             in_=Pd[:, 1280 + bh * 300:1280 + bh * 300 + 300],
                                             func=AF.Copy)
                        scatters.append((d, t, bh, osb))
                # scatters emitted after both dirs' elementwise so the GpSimd
                # FIFO head never blocks the update ops
                for d, tt_, bh, osb in scatters:
                    nc.gpsimd.indirect_dma_start(
                        out=out[:, :],
                        out_offset=bass.IndirectOffsetOnAxis(
                            ap=idx_sb[bh][:, tt_:tt_ + 1], axis=0),
                        in_=osb[:, :], in_offset=None,
                        element_offset=d * H,
                        bounds_check=breg, oob_is_err=False)

    _split_waits(nc)
    return nc


def _prep_weights(w_ih, w_hh, b_ih, b_hh):
    wT = np.ascontiguousarray(w_ih[PERM, :].T)          # [300, 900]
    hT = np.ascontiguousarray(w_hh[PERM, :].T)
    bi = b_ih[PERM]
    bh = b_hh[PERM]
    n_mask = PERM >= 600
    aug_x = np.where(n_mask, bi, 0.0)                   # b_ih for n via x ones-row
    aug_h = bh + np.where(~n_mask, bi, 0.0)             # b_hh (+ b_ih for r,z)
    bf = ml_dtypes.bfloat16
    return {
        "wk0": np.ascontiguousarray(wT[0:128]).astype(bf),
        "wk1": np.ascontiguousarray(wT[128:256]).astype(bf),
        "hk0": np.ascontiguousarray(hT[0:128]).astype(bf),
        "hk1": np.ascontiguousarray(hT[128:256]).astype(bf),
        "k2m": np.ascontiguousarray(
            np.vstack([hT[256:300], aug_h[None, :], np.zeros((19, H3), np.float32),
                       wT[256:300], aug_x[None, :], np.zeros((3, H3), np.float32)])
        ).astype(bf),
    }


TRACE = False
TRACE_DIR = None
LAST_EXEC_NS = None


def kernel(node, a_scope, max_len, bias, w_ih_f, w_hh_f, b_ih_f, b_hh_f,
           w_ih_b, w_hh_b, b_ih_b, b_hh_b):
    global LAST_EXEC_NS
    node = np.asarray(node, dtype=np.float32)
    bias_np = np.asarray(bias, dtype=np.float32)
    a_scope = np.asarray(a_scope, dtype=np.int64)
    assert int(max_len) == L and node.shape == (N, H) and a_scope.shape == (B,)

    ends = np.cumsum(a_scope)
    starts = ends - a_scope
    core_lo = starts[0::BC]                    # first row of each core's block
    core_hi = ends[BC - 1::BC]                 # end row of each core's block
    rows = (core_hi - core_lo).astype(np.int64)
    nc_cap = int(rows.max())

    key = nc_cap
    if key not in _cache:
        _cache[key] = _build(nc_cap)
    nc = _cache[key]

    wf = _prep_weights(np.asarray(w_ih_f), np.asarray(w_hh_f),
                       np.asarray(b_ih_f), np.asarray(b_hh_f))
    wb = _prep_weights(np.asarray(w_ih_b), np.asarray(w_hh_b),
                       np.asarray(b_ih_b), np.asarray(b_hh_b))
    negb = np.ascontiguousarray(-bias_np.reshape(H, 1))
    node_b = node + bias_np[None, :]           # fold bias host-side

    in_maps = []
    for c in range(NCORES):
        lo, hi = int(core_lo[c]), int(core_hi[c])
        node_c = np.zeros((nc_cap, H), dtype=np.float32)
        node_c[:hi - lo] = node_b[lo:hi]
        st = (starts[c * BC:(c + 1) * BC] - lo).astype(np.int64)
        ln = a_scope[c * BC:(c + 1) * BC]
        tt = np.arange(L, dtype=np.int64)
        im = st[:, None] + tt[None, :]                      # [BC, L]
        im = np.where(tt[None, :] < ln[:, None], im, OOB).astype(np.int32)
        ci = np.zeros((68, BC), dtype=ml_dtypes.bfloat16)
        ci[0] = 1.0   # comb row 44: ones (aug_h)
        ci[64] = 1.0  # comb row 108: ones (aug_x)
        m = {"node": node_c, "idx": np.ascontiguousarray(im),
             "negbv": negb, "cinit": ci}
        for d, wd in enumerate((wf, wb)):
            for nm in ("wk0", "wk1", "hk0", "hk1", "k2m"):
                m[f"{nm}_{d}"] = wd[nm]
        in_maps.append(m)

    res = run_bass_kernel_spmd(nc, in_maps, core_ids=list(range(NCORES)),
                               trace=TRACE, tmpdir=TRACE_DIR)
    LAST_EXEC_NS = res.exec_time_ns
    out = np.empty((N, 2 * H), dtype=np.float32)
    for c in range(NCORES):
        lo, hi = int(core_lo[c]), int(core_hi[c])
        out[lo:hi] = res.results[c]["out"][:hi - lo]
    return out


# revision 28
# speedup vs baseline: 1.1759x; 1.0135x over previous
"""Bidirectional batched GRU over ragged sequences on 8 Trainium2 NeuronCores.

Layout: hidden dim H=300 on partitions (3 chunks 128/128/44), batch on the
free dim. Per core: 256 segments, fwd+bwd scans interleaved. Biases enter via
an augmented ones-row in the matmul rhs. Outputs are PE-transposed (identity
matmul) and scattered to the flat ragged layout via indirect DMA.

v3: persistent PSUM accumulators (fine-grained instruction deps instead of
per-step pool-slot recycling), x-projection matmuls decoupled from the h
recurrence chain, 3-way split sigmoid, chunk-staged hidden update, bias folded
into the node tensor host-side. Segments are separated by 3 guard rows so one
single-offset indirect descriptor can move 4 consecutive positions (gather)
or 2 (scatter, into per-direction outputs concatenated host-side).
"""
import sys
sys.path.insert(0, "/opt/trn_rl_repo")
import numpy as np
import ml_dtypes

import concourse.bass as bass
import concourse.mybir as mybir
import concourse.tile as _tile_mod
from concourse.tile import TileContext
from concourse.tile_rust import add_dep_helper
from concourse.bass_utils import run_bass_kernel_spmd
from concourse.masks import make_identity

# ---- workaround: this walrus build rejects instructions carrying more than
# one semaphore wait. (a) distribute the TileContext tail-drain waits over
# single-wait SP no-ops; (b) post-pass hoisting excess waits anywhere else.
try:
    from bass_rust import ScopedClock as _ScopedClock
except ImportError:
    _ScopedClock = _tile_mod.ScopedClock


def _patched_drain_and_barrier(self, tick_clock, wait_clock):
    nc = self.nc
    probe = nc.sync.nop()
    wait_clock.add_sem_waits(probe.ins, _ScopedClock({None: tick_clock.global_clock}))
    si = probe.ins.sync_info
    waits = list(si.on_wait) if si is not None else []
    ups = list(si.on_update) if si is not None else []
    probe.ins.sync_info = mybir.SyncInfo(on_wait=[], on_update=ups)
    for w in waits:
        nc.sync.nop().ins.sync_info = mybir.SyncInfo(on_wait=[w], on_update=[])
    nc.sync.drain()
    nc.all_engine_barrier()
    assert self.sems is not None
    popped = nc._tile_sem_poison_stack.pop()
    assert popped is self._sem_poison
    nc.clear_and_free_semaphores(list(self.sems.allocated().values()))
    nc.all_engine_barrier()


TileContext._drain_and_barrier = _patched_drain_and_barrier
_nop_ctr = [0]


def _split_waits(nc, maxw=1):
    n_split = 0
    for fn in nc.m.functions:
        for bb in fn.blocks:
            il = bb.instructions
            newl = []
            for ins in il:
                si = ins.sync_info
                if si is not None and len(si.on_wait) > maxw:
                    waits = list(si.on_wait)
                    ups = list(si.on_update)
                    hoist, keep = waits[:-maxw], waits[-maxw:]
                    for i in range(0, len(hoist), maxw):
                        _nop_ctr[0] += 1
                        nop = mybir.InstNoOp(
                            name=f"waitnop-{_nop_ctr[0]}",
                            sync_info=mybir.SyncInfo(
                                on_wait=hoist[i:i + maxw], on_update=[]),
                            bass_nofuse=True,
                            engine=ins.engine)
                        nc.register_instruction(nop, overwrite=True)
                        newl.append(nop)
                    ins.sync_info = mybir.SyncInfo(on_wait=keep, on_update=ups)
                    n_split += 1
                newl.append(ins)
            il[:] = newl
    return n_split

F32 = mybir.dt.float32
BF16 = mybir.dt.bfloat16
I32 = mybir.dt.int32
AF = mybir.ActivationFunctionType
ALU = mybir.AluOpType

B, H, L, N = 2048, 300, 128, 131072
NCORES = 8
BC = B // NCORES          # 256 segments per core
H3 = 3 * H                # 900
HP = [(0, 128), (128, 256), (256, 300)]        # H chunks (partition dim)
# permuted gate-row order: [r0 r1 z0 z1 (r2 z2) n0 n1 n2]
PERM = np.concatenate([
    np.arange(0, 128), np.arange(128, 256),         # r0 r1
    np.arange(300, 428), np.arange(428, 556),       # z0 z1
    np.arange(256, 300), np.arange(556, 600),       # r2 z2  (Mc4, 88 rows)
    np.arange(600, 900),                            # n
])
MC = [(0, 128), (128, 256), (256, 384), (384, 512), (512, 600),
      (600, 728), (728, 856), (856, 900)]           # M chunks (permuted space)
OOB = 2 ** 30
GB = 4                    # gather batch (consecutive positions per descriptor)
GUARD = 3                 # guard rows between segments

_cache = {}


def _dep(after, before):
    # force same-engine stream order: `after` must issue after `before`
    add_dep_helper(after.ins, before.ins, sync=False,
                   reason="psum bank pending-zero order")


def _build(nc_cap):
    nc = bass.Bass()
    node = nc.dram_tensor("node", [nc_cap, H], F32, kind="ExternalInput")
    idx = nc.dram_tensor("idx", [BC, L], I32, kind="ExternalInput")
    negbv = nc.dram_tensor("negbv", [H, 1], F32, kind="ExternalInput")
    win = {}
    for d in range(2):
        for nm in ("wk0", "wk1", "hk0", "hk1"):
            win[(nm, d)] = nc.dram_tensor(f"{nm}_{d}", [128, H3], BF16,
                                          kind="ExternalInput")
        win[("k2m", d)] = nc.dram_tensor(f"k2m_{d}", [112, H3], BF16,
                                         kind="ExternalInput")
    cinit = nc.dram_tensor("cinit", [68, BC], BF16, kind="ExternalInput")
    msg = nc.dram_tensor("msg", [L, 128, 768], BF16)  # internal scratch
    out = nc.dram_tensor("out", [nc_cap, 2 * H], F32, kind="ExternalOutput")

    with TileContext(nc) as tc, \
         tc.tile_pool(name="persist", bufs=1) as pers:
        breg = nc.gpsimd.to_reg(nc_cap - 1)
        def ptile(shape, dtype, name):
            return pers.tile(shape, dtype, name=name, tag=name)
        consts = ptile([128, 128], F32, "consts")
        idf = consts[:, 0:128]
        make_identity(nc, idf)
        idb_t = ptile([128, 128], BF16, "idb")
        make_identity(nc, idb_t[:])
        negb_sb = ptile([128, 3], F32, "negb_sb")
        for c, (lo, hi) in enumerate(HP):
            nc.sync.dma_start(out=negb_sb[0:hi - lo, c:c + 1], in_=negbv[lo:hi, :])
        idx_sb = []
        for hh in range(2):
            t_ = ptile([128, L], I32, f"idx{hh}")
            nc.sync.dma_start(out=t_[:], in_=idx[hh * 128:(hh + 1) * 128, :])
            idx_sb.append(t_)
        # weights
        W = {}
        for d in range(2):
            for nm in ("wk0", "wk1", "hk0", "hk1"):
                t_ = ptile([128, H3], BF16, f"{nm}_{d}_sb")
                nc.sync.dma_start(out=t_[:], in_=win[(nm, d)][:])
                W[(nm, d)] = t_
            t_ = ptile([112, H3], BF16, f"k2m_{d}_sb")
            nc.sync.dma_start(out=t_[:], in_=win[("k2m", d)][:])
            W[("k2m", d)] = t_
        # persistent state
        h0acc = ptile([128, 768], F32, "h0acc")   # max(node+bias), chunk-major
        hk = {}   # (dir, chunk 0/1) -> [128, 256] bf16
        for d in range(2):
            for c in range(2):
                hk[(d, c)] = ptile([128, BC], BF16, f"h_{d}_{c}")
        comb = {}  # (dir, pingpong) -> [112, 256]: rows 0:44 h2, 44 ones, 64:108 x2, 108 ones
        for d in range(2):
            for pp in range(2):
                t_ = ptile([112, BC], BF16, f"comb_{d}_{pp}")
                nc.sync.dma_start(out=t_[44:112, :], in_=cinit[:])
                comb[(d, pp)] = t_

        # ---------------- Phase A: gather -> transpose -> relu -> msg slabs + h0
        with tc.tile_pool(name="gpool", bufs=4) as gpool, \
             tc.tile_pool(name="spsum", bufs=3, space="PSUM") as spsum, \
             tc.tile_pool(name="slabpool", bufs=3) as slabpool:
            for t in range(L):
                ps = spsum.tile([128, 768], F32, tag="ps")
                for hh in range(2):
                    g = gpool.tile([128, H], F32, tag=f"g{hh}")
                    nc.vector.memset(g[:], -1.0e30)
                    nc.gpsimd.indirect_dma_start(
                        out=g[:], out_offset=None, in_=node[:],
                        in_offset=bass.IndirectOffsetOnAxis(
                            ap=idx_sb[hh][:, t:t + 1], axis=0),
                        bounds_check=breg, oob_is_err=False)
                    for c, (lo, hi) in enumerate(HP):
                        nc.tensor.matmul(
                            out=ps[0:hi - lo, c * 256 + hh * 128: c * 256 + hh * 128 + 128],
                            lhsT=g[:, lo:hi], rhs=idf[0:128, 0:128],
                            start=True, stop=True)
                slab = slabpool.tile([128, 768], BF16, tag="slab_sb")
                nc.scalar.activation(out=slab[:], in_=ps[:], func=AF.Relu)
                if t == 0:
                    nc.vector.tensor_copy(out=h0acc[:], in_=ps[:])
                else:
                    nc.vector.tensor_tensor(out=h0acc[:], in0=h0acc[:],
                                            in1=ps[:], op=ALU.max)
                nc.sync.dma_start(out=msg[t], in_=slab[:])

        # h state init from h0acc (undo the folded +bias with ACT bias=-b)
        for d in range(2):
            for c in range(2):
                nc.scalar.activation(out=hk[(d, c)][:], in_=h0acc[:, c * 256:(c + 1) * 256],
                                     func=AF.Identity, bias=negb_sb[:, c:c + 1], scale=1.0)
            nc.scalar.activation(out=comb[(d, 0 if d == 0 else 1)][0:44, :],
                                 in_=h0acc[0:44, 512:768],
                                 func=AF.Identity, bias=negb_sb[0:44, 2:3], scale=1.0)

        # ---------------- Phase B: interleaved fwd/bwd scan
        with tc.tile_pool(name="pscan", bufs=1, space="PSUM") as pscan, \
             tc.tile_pool(name="xpool", bufs=4) as xpool, \
             tc.tile_pool(name="gates", bufs=3) as gates, \
             tc.tile_pool(name="opool", bufs=4) as opool:
            P = [pscan.tile([128, 2048], F32, name=f"P{d}", tag=f"P{d}")
                 for d in range(2)]
            # region -> psum column map: each bank holds one rz region (whose
            # first matmul start=True marks the bank's pending-zero bits) and
            # one nh region (start=False, riding the sibling's mark).
            PCOL_RZ = [0, 512, 1024, 1536, 1792]      # r0 r1 z0 z1 (r2z2)
            PCOL_NH = [256, 768, 1280]                # n0 n1 n2
            PCOL_XN = [0, 512, 1024]                  # xn overlays r0 r1 z0
            OUTCOL = 1280                             # out overlays n2 z1 E[0:88]
            for s in range(L):
                scatters = []
                for d in range(2):
                    t = s if d == 0 else L - 1 - s
                    Pd = P[d]
                    cb = comb[(d, t % 2)]
                    cbn = comb[(d, (t + 1) % 2)]
                    xk0 = xpool.tile([128, BC], BF16, tag="xk0")
                    xk1 = xpool.tile([128, BC], BF16, tag="xk1")
                    nc.sync.dma_start(out=xk0[:], in_=msg[t, :, 0:256])
                    nc.sync.dma_start(out=xk1[:], in_=msg[t, :, 256:512])
                    nc.sync.dma_start(out=cb[64:108, :], in_=msg[t, 0:44, 512:768])
                    wk0, wk1 = W[("wk0", d)], W[("wk1", d)]
                    hk0w, hk1w = W[("hk0", d)], W[("hk1", d)]
                    k2m = W[("k2m", d)]
                    h0t, h1t = hk[(d, 0)], hk[(d, 1)]
                    # --- x-side rz accumulation (start=True marks each bank).
                    # E (j=4) shares bank3 with z1 (j=3): its whole group is
                    # emitted after z1's group (bank writes execute in program
                    # order), so its bank-remark can't wipe z1 mid-group.
                    g1first = {}
                    g1last = {}
                    for j in range(4):
                        lo, hi = MC[j]
                        m = hi - lo
                        o = Pd[0:m, PCOL_RZ[j]:PCOL_RZ[j] + BC]
                        mm = nc.tensor.matmul(out=o, lhsT=wk0[:, lo:hi], rhs=xk0[:],
                                              start=True, stop=False,
                                              skip_group_check=True)
                        g1first[j] = mm
                        nc.tensor.matmul(out=o, lhsT=wk1[:, lo:hi], rhs=xk1[:],
                                         start=False, stop=False,
                                         skip_group_check=True)
                    # --- h-side rz accumulation
                    for j in range(4):
                        lo, hi = MC[j]
                        m = hi - lo
                        o = Pd[0:m, PCOL_RZ[j]:PCOL_RZ[j] + BC]
                        nc.tensor.matmul(out=o, lhsT=hk0w[:, lo:hi], rhs=h0t[:],
                                         start=False, stop=False,
                                         skip_group_check=True)
                        nc.tensor.matmul(out=o, lhsT=hk1w[:, lo:hi], rhs=h1t[:],
                                         start=False, stop=False,
                                         skip_group_check=True)
                        g1last[j] = nc.tensor.matmul(
                            out=o, lhsT=k2m[0:109, lo:hi], rhs=cb[0:109, :],
                            start=False, stop=True, skip_group_check=True)
                    # region E full group after z1's group
                    lo, hi = MC[4]
                    m = hi - lo
                    o = Pd[0:m, PCOL_RZ[4]:PCOL_RZ[4] + BC]
                    mm = nc.tensor.matmul(out=o, lhsT=wk0[:, lo:hi], rhs=xk0[:],
                                          start=True, stop=False,
                                          skip_group_check=True)
                    g1first[4] = mm
                    _dep(mm, g1last[3])
                    nc.tensor.matmul(out=o, lhsT=wk1[:, lo:hi], rhs=xk1[:],
                                     start=False, stop=False, skip_group_check=True)
                    nc.tensor.matmul(out=o, lhsT=hk0w[:, lo:hi], rhs=h0t[:],
                                     start=False, stop=False, skip_group_check=True)
                    nc.tensor.matmul(out=o, lhsT=hk1w[:, lo:hi], rhs=h1t[:],
                                     start=False, stop=False, skip_group_check=True)
                    g1last[4] = nc.tensor.matmul(
                        out=o, lhsT=k2m[0:109, lo:hi], rhs=cb[0:109, :],
                        start=False, stop=True, skip_group_check=True)
                    # --- nh regions ride their bank-sibling rz region's mark
                    g2first = {}
                    g2last = {}
                    for jj in range(3):
                        lo, hi = MC[5 + jj]
                        m = hi - lo
                        o = Pd[0:m, PCOL_NH[jj]:PCOL_NH[jj] + BC]
                        mm = nc.tensor.matmul(out=o, lhsT=hk0w[:, lo:hi], rhs=h0t[:],
                                              start=False, stop=False,
                                              skip_group_check=True)
                        g2first[jj] = mm
                        _dep(mm, g1first[jj])
                        nc.tensor.matmul(out=o, lhsT=hk1w[:, lo:hi], rhs=h1t[:],
                                         start=False, stop=False,
                                         skip_group_check=True)
                        g2last[jj] = nc.tensor.matmul(
                            out=o, lhsT=k2m[0:45, lo:hi], rhs=cb[0:45, :],
                            start=False, stop=True, skip_group_check=True)
                    # --- gates: split sigmoid so r0/r1/(r2z2) unblock tmp
                    # early. rz sbuf layout: r0 r1 z0 z1 E at 256-col slots.
                    rz = gates.tile([128, 1280], BF16, tag=f"rz{d}")
                    # order: r0 first (tmp chunk0), z0 second (unblocks xn2 →
                    # ssb), then r1, then z1+E merged
                    nc.scalar.activation(out=rz[:, 0:256], in_=Pd[:, 0:256],
                                         func=AF.Sigmoid)
                    nc.scalar.activation(out=rz[:, 512:768], in_=Pd[:, 1024:1280],
                                         func=AF.Sigmoid)
                    nc.scalar.activation(out=rz[:, 256:512], in_=Pd[:, 512:768],
                                         func=AF.Sigmoid)
                    nc.scalar.activation(out=rz[:, 768:1280], in_=Pd[:, 1536:2048],
                                         func=AF.Sigmoid)
                    # z2 partition shift 44:88 -> 0:44 (E sbuf slot is 1024:1280)
                    z2c = gates.tile([44, BC], BF16, tag=f"z2{d}")
                    nc.sync.dma_start(out=z2c[0:44, :], in_=rz[44:88, 1024:1280])
                    # --- xn into freed rz regions (re-marks banks 0,1,2 after
                    # both sibling groups completed)
                    xn_last = {}
                    for jj in range(3):
                        lo, hi = MC[5 + jj]
                        m = hi - lo
                        o = Pd[0:m, PCOL_XN[jj]:PCOL_XN[jj] + BC]
                        mm = nc.tensor.matmul(out=o, lhsT=wk0[:, lo:hi], rhs=xk0[:],
                                              start=True, stop=False,
                                              skip_group_check=True)
                        _dep(mm, g1last[jj])
                        _dep(mm, g2last[jj])
                        nc.tensor.matmul(out=o, lhsT=wk1[:, lo:hi], rhs=xk1[:],
                                         start=False, stop=False,
                                         skip_group_check=True)
                        xn_last[jj] = nc.tensor.matmul(
                            out=o, lhsT=k2m[64:109, lo:hi], rhs=cb[64:109, :],
                            start=False, stop=True, skip_group_check=True)
                    # tmp = r * nh ; s = tmp + xn ; n = tanh(s)
                    tmp = gates.tile([128, 768], F32, tag=f"tmp{d}")
                    rsl = [rz[:, 0:256], rz[:, 256:512], rz[0:44, 1024:1280]]
                    for c, (lo, hi) in enumerate(HP):
                        r = hi - lo
                        nc.vector.tensor_tensor(
                            out=tmp[0:r, c * 256:(c + 1) * 256], in0=rsl[c],
                            in1=Pd[0:r, PCOL_NH[c]:PCOL_NH[c] + BC],
                            op=ALU.mult)
                    ssb = gates.tile([128, 768], F32, tag=f"s{d}")
                    for c, (lo, hi) in enumerate(HP):
                        r = hi - lo
                        nc.vector.tensor_tensor(
                            out=ssb[0:r, c * 256:(c + 1) * 256],
                            in0=tmp[0:r, c * 256:(c + 1) * 256],
                            in1=Pd[0:r, PCOL_XN[c]:PCOL_XN[c] + BC],
                            op=ALU.add)
                    nsb = gates.tile([128, 768], BF16, tag=f"n{d}")
                    nc.scalar.activation(out=nsb[:], in_=ssb[:], func=AF.Tanh)
                    # h' = n + z*(h-n), chunk-staged (chunk 0 fully on DVE so
                    # the next step's hk0 matmuls unblock earliest)
                    zsl = [rz[:, 512:768], rz[:, 768:1024], z2c[0:44, :]]
                    hsl = [h0t[:], h1t[:], cb[0:44, :]]
                    hnx = [h0t[:], h1t[:], cbn[0:44, :]]
                    for c, (lo, hi) in enumerate(HP):
                        r = hi - lo
                        ns = nsb[0:r, c * 256:(c + 1) * 256]
                        dd = gates.tile([128, BC], BF16, tag=f"d{d}")
                        nc.vector.tensor_tensor(out=dd[0:r, :],
                                                in0=hsl[c][0:r, :] if c == 2 else hsl[c],
                                                in1=ns, op=ALU.subtract)
                        ee = gates.tile([128, BC], BF16, tag=f"e{d}")
                        eng = nc.vector if c == 0 else nc.gpsimd
                        eng.tensor_tensor(out=ee[0:r, :],
                                          in0=zsl[c][0:r, :] if c == 2 else zsl[c],
                                          in1=dd[0:r, :], op=ALU.mult)
                        nc.vector.tensor_tensor(out=hnx[c][0:r, :] if c == 2 else hnx[c],
                                                in0=ns, in1=ee[0:r, :], op=ALU.add)
                    # transpose h' to [b, H] in psum cols 1280:1880, write into
                    # the 2-step osb pair buffer; scatter fires every 2nd step
                    hpieces = [h0t, h1t, cbn]
                    for bh in range(2):
                        for c, (lo, hi) in enumerate(HP):
                            r = hi - lo
                            src = hpieces[c]
                            lhsT = (src[0:44, bh * 128:(bh + 1) * 128] if c == 2
                                    else src[:, bh * 128:(bh + 1) * 128])
                            trmm = nc.tensor.matmul(
                                out=Pd[:, 1280 + bh * 300 + lo:1280 + bh * 300 + hi],
                                lhsT=lhsT, rhs=idb_t[0:r, 0:r],
                                start=True, stop=True, skip_group_check=True)
                            if bh == 0 and c <= 1:
                                # bank2 pending-mark must not precede xn2's
                                # accumulation (no AP overlap to order them)
                                _dep(trmm, xn_last[2])
                        scatters.append((d, t, bh, Pd))
                # osb copies + scatters after both dirs' chains so neither the
                # ACT nor the GpSimd FIFO head blocks the other direction
                for d, tt_, bh, Pd in scatters:
                    osb = opool.tile([128, H], F32, tag="osb",
                                     name=f"osb{d}_{s}_{bh}")
                    nc.scalar.activation(out=osb[:],
                                         in_=Pd[:, 1280 + bh * 300:1280 + bh * 300 + 300],
                                         func=AF.Copy)
                    nc.gpsimd.indirect_dma_start(
                        out=out[:, :],
                        out_offset=bass.IndirectOffsetOnAxis(
                            ap=idx_sb[bh][:, tt_:tt_ + 1], axis=0),
                        in_=osb[:, :], in_offset=None,
                        element_offset=d * H,
                        bounds_check=breg, oob_is_err=False)

    _split_waits(nc)
    return nc


def _prep_weights(w_ih, w_hh, b_ih, b_hh):
    wT = np.ascontiguousarray(w_ih[PERM, :].T)          # [300, 900]
    hT = np.ascontiguousarray(w_hh[PERM, :].T)
    bi = b_ih[PERM]
    bh = b_hh[PERM]
    n_mask = PERM >= 600
    aug_x = np.where(n_mask, bi, 0.0)                   # b_ih for n via x ones-row
    aug_h = bh + np.where(~n_mask, bi, 0.0)             # b_hh (+ b_ih for r,z)
    bf = ml_dtypes.bfloat16
    return {
        "wk0": np.ascontiguousarray(wT[0:128]).astype(bf),
        "wk1": np.ascontiguousarray(wT[128:256]).astype(bf),
        "hk0": np.ascontiguousarray(hT[0:128]).astype(bf),
        "hk1": np.ascontiguousarray(hT[128:256]).astype(bf),
        "k2m": np.ascontiguousarray(
            np.vstack([hT[256:300], aug_h[None, :], np.zeros((19, H3), np.float32),
                       wT[256:300], aug_x[None, :], np.zeros((3, H3), np.float32)])
        ).astype(bf),
    }


TRACE = False
TRACE_DIR = None
LAST_EXEC_NS = None


def kernel(node, a_scope, max_len, bias, w_ih_f, w_hh_f, b_ih_f, b_hh_f,
           w_ih_b, w_hh_b, b_ih_b, b_hh_b):
    global LAST_EXEC_NS
    node = np.asarray(node, dtype=np.float32)
    bias_np = np.asarray(bias, dtype=np.float32)
    a_scope = np.asarray(a_scope, dtype=np.int64)
    assert int(max_len) == L and node.shape == (N, H) and a_scope.shape == (B,)

    ends = np.cumsum(a_scope)
    starts = ends - a_scope
    core_lo = starts[0::BC]                    # first row of each core's block
    core_hi = ends[BC - 1::BC]                 # end row of each core's block
    rows = (core_hi - core_lo).astype(np.int64)
    nc_cap = int(rows.max())

    key = nc_cap
    if key not in _cache:
        _cache[key] = _build(nc_cap)
    nc = _cache[key]

    wf = _prep_weights(np.asarray(w_ih_f), np.asarray(w_hh_f),
                       np.asarray(b_ih_f), np.asarray(b_hh_f))
    wb = _prep_weights(np.asarray(w_ih_b), np.asarray(w_hh_b),
                       np.asarray(b_ih_b), np.asarray(b_hh_b))
    negb = np.ascontiguousarray(-bias_np.reshape(H, 1))
    node_b = node + bias_np[None, :]           # fold bias host-side

    in_maps = []
    for c in range(NCORES):
        lo, hi = int(core_lo[c]), int(core_hi[c])
        node_c = np.zeros((nc_cap, H), dtype=np.float32)
        node_c[:hi - lo] = node_b[lo:hi]
        st = (starts[c * BC:(c + 1) * BC] - lo).astype(np.int64)
        ln = a_scope[c * BC:(c + 1) * BC]
        tt = np.arange(L, dtype=np.int64)
        im = st[:, None] + tt[None, :]                      # [BC, L]
        im = np.where(tt[None, :] < ln[:, None], im, OOB).astype(np.int32)
        ci = np.zeros((68, BC), dtype=ml_dtypes.bfloat16)
        ci[0] = 1.0   # comb row 44: ones (aug_h)
        ci[64] = 1.0  # comb row 108: ones (aug_x)
        m = {"node": node_c, "idx": np.ascontiguousarray(im),
             "negbv": negb, "cinit": ci}
        for d, wd in enumerate((wf, wb)):
            for nm in ("wk0", "wk1", "hk0", "hk1", "k2m"):
                m[f"{nm}_{d}"] = wd[nm]
        in_maps.append(m)

    res = run_bass_kernel_spmd(nc, in_maps, core_ids=list(range(NCORES)),
                               trace=TRACE, tmpdir=TRACE_DIR)
    LAST_EXEC_NS = res.exec_time_ns
    out = np.empty((N, 2 * H), dtype=np.float32)
    for c in range(NCORES):
        lo, hi = int(core_lo[c]), int(core_hi[c])
        out[lo:hi] = res.results[c]["out"][:hi - lo]
    return out
